# revision 8
# baseline (speedup 1.0000x reference)
"""Bidirectional cross-attention Trainium2 Bass kernel.

Data-parallel over batch: 8 cores, one batch element each (SPMD, no
collectives).  Per core:
  h1T = (x1 @ W_p1 + b_p1).T   [512, 2048]  (f32r matmuls, fp32 PE transposes)
  h2T = (x2 @ W_p2 + b_p2).T
  k/v projections to bf16 (kT transposed layout, v natural layout)
  flash-style attention per 128-row q tile, both directions interleaved:
    scores (bf16 matmul) -> exp on ACT (scale folded in, accum_out row sums)
    -> PE-transpose probs -> probs @ V -> scale by gamma/rowsum
  epilogue: out = gamma*(f1+f2) + h1 + h2 via accumulating fp32 PE transposes
Softmax max-subtraction is omitted: scores = q.k/sqrt(512) with these
projection scales is O(1), far from exp overflow.
"""
import sys

if "/opt/trn_rl_repo" not in sys.path:
    sys.path.insert(0, "/opt/trn_rl_repo")

import numpy as np

S = 2048
D = 512
K1 = 768
K2 = 1024
N_CORES = 8
SCALE = 1.0 / float(np.sqrt(np.float32(D)))

_BUILD_CACHE = {}


def build_bass(reps=None):
    """Build (and cache) the compiled Bass program.  reps=None -> single shot;
    reps=int -> whole kernel body wrapped in a hardware For_i loop (used only
    for benchmarking)."""
    if reps in _BUILD_CACHE:
        return _BUILD_CACHE[reps]

    import concourse.bass as bass
    from concourse import bacc
    import concourse.mybir as mybir
    import concourse.tile as tile
    from concourse.masks import make_identity
    from contextlib import ExitStack

    F32 = mybir.dt.float32
    F32R = mybir.dt.float32r
    BF16 = mybir.dt.bfloat16
    AF = mybir.ActivationFunctionType

    nc = bacc.Bacc(None, target_bir_lowering=False, debug=False)

    x1 = nc.dram_tensor("x1", [S, K1], F32, kind="ExternalInput")
    x2 = nc.dram_tensor("x2", [S, K2], F32, kind="ExternalInput")
    W_p1 = nc.dram_tensor("W_p1", [K1, D], F32R, kind="ExternalInput")
    W_p2 = nc.dram_tensor("W_p2", [K2, D], F32R, kind="ExternalInput")
    W_q = nc.dram_tensor("W_q", [D, D], F32R, kind="ExternalInput")
    W_k = nc.dram_tensor("W_k", [D, D], F32R, kind="ExternalInput")
    W_v = nc.dram_tensor("W_v", [D, D], F32R, kind="ExternalInput")
    b_p1 = nc.dram_tensor("b_p1", [D], F32, kind="ExternalInput")
    b_p2 = nc.dram_tensor("b_p2", [D], F32, kind="ExternalInput")
    b_q = nc.dram_tensor("b_q", [D], F32, kind="ExternalInput")
    b_k = nc.dram_tensor("b_k", [D], F32, kind="ExternalInput")
    b_v = nc.dram_tensor("b_v", [D], F32, kind="ExternalInput")
    gamma = nc.dram_tensor("gamma", [1], F32, kind="ExternalInput")
    out = nc.dram_tensor("out", [S, D], F32, kind="ExternalOutput")

    with tile.TileContext(nc) as tc, ExitStack() as top:
        const = top.enter_context(tc.tile_pool(name="const", bufs=1))
        persist = top.enter_context(tc.tile_pool(name="persist", bufs=1))
        ps_small = top.enter_context(tc.tile_pool(name="ps_small", bufs=2, space="PSUM"))
        ps_mid = top.enter_context(tc.tile_pool(name="ps_mid", bufs=2, space="PSUM"))
        ps_big = top.enter_context(tc.tile_pool(name="ps_big", bufs=2, space="PSUM"))

        def body():
            # ---- constants ----
            identf = const.tile([128, 128], F32, tag="identf")
            make_identity(nc, identf[:])
            identb = const.tile([128, 128], BF16, tag="identb")
            make_identity(nc, identb[:])
            gam = const.tile([128, 1], F32, tag="gam")
            nc.sync.dma_start(gam[:], bass.AP(tensor=gamma, offset=0, ap=[[0, 128], [1, 1]]))
            bp1 = const.tile([128, 4], F32, tag="bp1")
            nc.sync.dma_start(bp1[:], b_p1.rearrange("(o p) -> p o", p=128))
            bp2 = const.tile([128, 4], F32, tag="bp2")
            nc.sync.dma_start(bp2[:], b_p2.rearrange("(o p) -> p o", p=128))
            bqt = const.tile([128, 4], F32, tag="bqt")
            nc.sync.dma_start(bqt[:], b_q.rearrange("(o p) -> p o", p=128))
            bkt = const.tile([128, 4], F32, tag="bkt")
            nc.sync.dma_start(bkt[:], b_k.rearrange("(o p) -> p o", p=128))
            bvr = const.tile([128, D], F32, tag="bvr")
            nc.sync.dma_start(bvr[:], bass.AP(tensor=b_v, offset=0, ap=[[0, 128], [1, D]]))

            h1T = persist.tile([128, 4, S], F32R, tag="h1T")
            h2T = persist.tile([128, 4, S], F32R, tag="h2T")
            Wq_s = persist.tile([128, 4, D], F32R, tag="Wq")
            nc.sync.dma_start(Wq_s[:], W_q.rearrange("(ko p) d -> p ko d", p=128))

            # ================= phase A: h1T / h2T =================
            with ExitStack() as phA:
                wp = phA.enter_context(tc.tile_pool(name="wp", bufs=1))
                xa = phA.enter_context(tc.tile_pool(name="xa", bufs=2))
                xb = phA.enter_context(tc.tile_pool(name="xb", bufs=1))
                Wp1_s = wp.tile([128, 6, D], F32R, tag="Wp1")
                nc.sync.dma_start(Wp1_s[:], W_p1.rearrange("(ko p) d -> p ko d", p=128))
                Wp2_s = wp.tile([128, 8, D], F32R, tag="Wp2")
                nc.sync.dma_start(Wp2_s[:], W_p2.rearrange("(ko p) d -> p ko d", p=128))

                for x_d, K, Wp_s, bt, hT in (
                    (x1, K1, Wp1_s, bp1, h1T),
                    (x2, K2, Wp2_s, bp2, h2T),
                ):
                    KC = K // 128
                    for sc in range(4):  # 512-wide s chunks
                        xin = xa.tile([128, 4, K2], F32, tag="xin")
                        nc.sync.dma_start(
                            xin[:, :, :K],
                            x_d[sc * 512:(sc + 1) * 512, :].rearrange(
                                "(ss p) k -> p ss k", p=128),
                        )
                        xT = xb.tile([128, 8, 512], F32R, tag="xT")
                        for kc in range(KC):
                            pt = ps_small.tile([128, 512], F32, tag="tp")
                            for ss in range(4):
                                nc.tensor.matmul(
                                    pt[:, ss * 128:(ss + 1) * 128],
                                    xin[:, ss, kc * 128:(kc + 1) * 128],
                                    identf[:], is_transpose=True,
                                    start=True, stop=True)
                            nc.vector.tensor_copy(xT[:, kc, :], pt[:])
                        for ds in range(4):
                            ph = ps_mid.tile([128, 512], F32, tag="mid")
                            for kc in range(KC):
                                nc.tensor.matmul(
                                    ph[:], Wp_s[:, kc, ds * 128:(ds + 1) * 128],
                                    xT[:, kc, :],
                                    start=(kc == 0), stop=(kc == KC - 1))
                            nc.vector.tensor_scalar_add(
                                hT[:, ds, sc * 512:(sc + 1) * 512], ph[:],
                                bt[:, ds:ds + 1])

            # ================= phase B: k/v projections =================
            phBC = ExitStack()
            kvpool = phBC.enter_context(tc.tile_pool(name="kvpool", bufs=1))
            k1T = kvpool.tile([128, 4, S], BF16, tag="k1T")
            k2T = kvpool.tile([128, 4, S], BF16, tag="k2T")
            v1 = kvpool.tile([128, 16, D], BF16, tag="v1")
            v2 = kvpool.tile([128, 16, D], BF16, tag="v2")
            with ExitStack() as phB:
                wkv = phB.enter_context(tc.tile_pool(name="wkv", bufs=1))
                Wk_s = wkv.tile([128, 4, D], F32R, tag="Wk")
                nc.sync.dma_start(Wk_s[:], W_k.rearrange("(ko p) d -> p ko d", p=128))
                Wv_s = wkv.tile([128, 4, D], F32R, tag="Wv")
                nc.sync.dma_start(Wv_s[:], W_v.rearrange("(ko p) d -> p ko d", p=128))

                for hT, kT in ((h1T, k1T), (h2T, k2T)):
                    for sc in range(4):
                        for ds in range(4):
                            ph = ps_mid.tile([128, 512], F32, tag="mid")
                            for kc in range(4):
                                nc.tensor.matmul(
                                    ph[:], Wk_s[:, kc, ds * 128:(ds + 1) * 128],
                                    hT[:, kc, sc * 512:(sc + 1) * 512],
                                    start=(kc == 0), stop=(kc == 3))
                            nc.vector.tensor_scalar_add(
                                kT[:, ds, sc * 512:(sc + 1) * 512], ph[:],
                                bkt[:, ds:ds + 1])
                for hT, vv in ((h1T, v1), (h2T, v2)):
                    for ss in range(16):
                        ph = ps_mid.tile([128, 512], F32, tag="mid")
                        for kc in range(4):
                            nc.tensor.matmul(
                                ph[:], hT[:, kc, ss * 128:(ss + 1) * 128],
                                Wv_s[:, kc, :],
                                start=(kc == 0), stop=(kc == 3))
                        nc.vector.tensor_add(vv[:, ss, :], bvr[:], ph[:])

            # ================= phase C: attention + epilogue =================
            with ExitStack() as phC:
                qp = phC.enter_context(tc.tile_pool(name="qp", bufs=1))
                pr = phC.enter_context(tc.tile_pool(name="pr", bufs=3))
                prT = phC.enter_context(tc.tile_pool(name="prT", bufs=2))
                sm = phC.enter_context(tc.tile_pool(name="sm", bufs=4))
                fo = phC.enter_context(tc.tile_pool(name="fo", bufs=2))

                q1g = q2g = None
                for j in range(16):  # 128-row q tiles
                    jj = j % 4
                    J = slice(j * 128, (j + 1) * 128)
                    if jj == 0:
                        g = j // 4
                        G = slice(g * 512, (g + 1) * 512)
                        q1g = qp.tile([128, 4, 512], BF16, tag="q1g")
                        q2g = qp.tile([128, 4, 512], BF16, tag="q2g")
                        for qg, hT in ((q1g, h1T), (q2g, h2T)):
                            for ds in range(4):
                                pq = ps_mid.tile([128, 512], F32, tag="mid")
                                for kc in range(4):
                                    nc.tensor.matmul(
                                        pq[:], Wq_s[:, kc, ds * 128:(ds + 1) * 128],
                                        hT[:, kc, G],
                                        start=(kc == 0), stop=(kc == 3))
                                nc.vector.tensor_scalar_add(
                                    qg[:, ds, :], pq[:], bqt[:, ds:ds + 1])

                    # --- scores + exp, both directions ---
                    probs = []
                    invs = []
                    for qg, kT in ((q1g, k2T), (q2g, k1T)):
                        pb = pr.tile([128, S], BF16, tag="probs")
                        accs = []
                        for h in range(2):
                            psS = ps_big.tile([128, 1024], F32, tag="big")
                            for kq in range(2):
                                for kc in range(4):
                                    nc.tensor.matmul(
                                        psS[:, kq * 512:(kq + 1) * 512],
                                        qg[:, kc, jj * 128:(jj + 1) * 128],
                                        kT[:, kc, h * 1024 + kq * 512:
                                           h * 1024 + (kq + 1) * 512],
                                        start=(kc == 0), stop=(kc == 3))
                            acc = sm.tile([128, 1], F32, tag="acc")
                            nc.scalar.activation(
                                pb[:, h * 1024:(h + 1) * 1024], psS[:],
                                AF.Exp, scale=SCALE, accum_out=acc[:])
                            accs.append(acc)
                        ssum = sm.tile([128, 1], F32, tag="ssum")
                        nc.vector.tensor_add(ssum[:], accs[0][:], accs[1][:])
                        rinv = sm.tile([128, 1], F32, tag="rinv")
                        nc.vector.reciprocal(rinv[:], ssum[:])
                        ginv = sm.tile([128, 1], F32, tag="ginv")
                        nc.vector.tensor_mul(ginv[:], rinv[:], gam[:])
                        probs.append(pb)
                        invs.append(ginv)

                    # --- transpose probs + PV, both directions ---
                    f_tile = None
                    for d_i, (pb, vv) in enumerate(((probs[0], v2), (probs[1], v1))):
                        pT = prT.tile([128, 16, 128], BF16, tag="pT")
                        for kc4 in range(4):
                            ptp = ps_small.tile([128, 512], BF16, tag="tp")
                            for ks in range(4):
                                kc = kc4 * 4 + ks
                                nc.tensor.matmul(
                                    ptp[:, ks * 128:(ks + 1) * 128],
                                    pb[:, kc * 128:(kc + 1) * 128],
                                    identb[:], is_transpose=True,
                                    start=True, stop=True)
                            nc.vector.tensor_copy(
                                pT[:, kc4 * 4:(kc4 + 1) * 4, :],
                                ptp[:].rearrange("p (a b) -> p a b", a=4))
                        psF = ps_mid.tile([128, 512], F32, tag="mid")
                        for kc in range(16):
                            nc.tensor.matmul(
                                psF[:], pT[:, kc, :], vv[:, kc, :],
                                start=(kc == 0), stop=(kc == 15))
                        if d_i == 0:
                            f_tile = fo.tile([128, 512], F32, tag="f")
                            nc.vector.tensor_scalar_mul(f_tile[:], psF[:], invs[0][:])
                        else:
                            f2 = fo.tile([128, 512], F32, tag="f2")
                            nc.vector.tensor_scalar_mul(f2[:], psF[:], invs[1][:])
                            nc.vector.tensor_add(f_tile[:], f_tile[:], f2[:])

                    # --- epilogue: out = f + (h1+h2).T ---
                    psH = ps_mid.tile([128, 512], F32, tag="mid")
                    for dc in range(4):
                        nc.tensor.matmul(
                            psH[:, dc * 128:(dc + 1) * 128],
                            h1T[:, dc, J].bitcast(F32), identf[:],
                            is_transpose=True, start=True, stop=False)
                        nc.tensor.matmul(
                            psH[:, dc * 128:(dc + 1) * 128],
                            h2T[:, dc, J].bitcast(F32), identf[:],
                            is_transpose=True, start=False, stop=True)
                    o_tile = fo.tile([128, 512], F32, tag="o")
                    nc.vector.tensor_add(o_tile[:], f_tile[:], psH[:])
                    nc.sync.dma_start(out[J, :], o_tile[:])
            phBC.close()

        if reps is None:
            body()
        else:
            import concourse.mybir as _mybir
            with tc.For_i(0, reps, 1, hint_engines=tuple(_mybir.ALL_ENGINES)):
                body()

    nc.compile()
    _BUILD_CACHE[reps] = nc
    return nc


def kernel(**inputs):
    from concourse.bass_utils import run_bass_kernel_spmd

    nc = build_bass(None)
    arrs = {k: np.asarray(v, dtype=np.float32) for k, v in inputs.items()}
    shared = {k: arrs[k] for k in
              ("W_p1", "W_p2", "W_q", "W_k", "W_v",
               "b_p1", "b_p2", "b_q", "b_k", "b_v", "gamma")}
    in_maps = [
        {"x1": arrs["x1"][b], "x2": arrs["x2"][b], **shared}
        for b in range(N_CORES)
    ]
    res = run_bass_kernel_spmd(nc, in_maps, list(range(N_CORES)))
    return np.stack([res.results[b]["out"] for b in range(N_CORES)], axis=0)


# revision 14
# speedup vs baseline: 1.2273x; 1.2273x over previous
"""Bidirectional cross-attention Trainium2 Bass kernel (v2).

Data-parallel over batch: 8 NeuronCores, one batch element each (SPMD, no
collectives).  Per core:

  phase A (fp32/f32r, PE transposes allowed):
    xT = PE-transpose(x) chunks; h = x @ W_p + b_p via f32r matmuls.
    Stored as hsumT = h1T+h2T (fp32, for the residual epilogue) and
    h1/h2 in fp8e4 (feeds the attention branch, which is scaled by gamma).
  phase B (fp8e4 DoubleRow matmuls):
    q1T,q2T,k1T,k2T (transposed [d,s]) and v1,v2 (natural [s,d]) in fp8e4.
  phase C (fp8e4 DoubleRow):
    scoresT[kpos,q] accumulated in PSUM -> exp on ACT (scale folded) ->
    probsT fp8 directly (no transposes!); row sums via ones-matmul
    (replicated across partitions); fusedT[d,q] = v.T @ probsT;
    normalize by gamma/sum (DVE).
  epilogue (PE transposes):
    outT = fusedT_1 + fusedT_2 + hsumT; PE-transpose to natural; DMA out.

The PE has a hardware hazard: a DoubleRow matmul adjacent to a transpose
matmul wedges the exec unit.  All DR work is fenced from transpose work by
dummy plain matmuls with explicit scheduling edges, and build() verifies
the final PE instruction order has no unfenced transitions.

Softmax max-subtraction is omitted: scores = q.k/sqrt(512) with these
projection scales is O(1), far from exp overflow.
"""
import sys

if "/opt/trn_rl_repo" not in sys.path:
    sys.path.insert(0, "/opt/trn_rl_repo")

import numpy as np

S = 2048
D = 512
K1 = 768
K2 = 1024
N_CORES = 8
SCALE = 1.0 / float(np.sqrt(np.float32(D)))

_BUILD_CACHE = {}


def build_bass(reps=None):
    if reps in _BUILD_CACHE:
        return _BUILD_CACHE[reps]

    import concourse.bass as bass
    from concourse import bacc
    import concourse.mybir as mybir
    import concourse.tile as tile
    from concourse.tile import add_dep_helper
    from concourse.masks import make_identity
    from contextlib import ExitStack

    F32 = mybir.dt.float32
    F32R = mybir.dt.float32r
    FP8 = mybir.dt.float8e4
    AF = mybir.ActivationFunctionType
    DR = mybir.MatmulPerfMode.DoubleRow

    nc = bacc.Bacc(None, target_bir_lowering=False, debug=False)

    x1 = nc.dram_tensor("x1", [S, K1], F32, kind="ExternalInput")
    x2 = nc.dram_tensor("x2", [S, K2], F32, kind="ExternalInput")
    W_p1 = nc.dram_tensor("W_p1", [K1, D], F32R, kind="ExternalInput")
    W_p2 = nc.dram_tensor("W_p2", [K2, D], F32R, kind="ExternalInput")
    W_q = nc.dram_tensor("W_q", [D, D], F32, kind="ExternalInput")
    W_k = nc.dram_tensor("W_k", [D, D], F32, kind="ExternalInput")
    W_v = nc.dram_tensor("W_v", [D, D], F32, kind="ExternalInput")
    b_p1 = nc.dram_tensor("b_p1", [D], F32, kind="ExternalInput")
    b_p2 = nc.dram_tensor("b_p2", [D], F32, kind="ExternalInput")
    b_q = nc.dram_tensor("b_q", [D], F32, kind="ExternalInput")
    b_k = nc.dram_tensor("b_k", [D], F32, kind="ExternalInput")
    b_v = nc.dram_tensor("b_v", [D], F32, kind="ExternalInput")
    gamma = nc.dram_tensor("gamma", [1], F32, kind="ExternalInput")
    out = nc.dram_tensor("out", [S, D], F32, kind="ExternalOutput")

    with tile.TileContext(nc) as tc, ExitStack() as top:
        const = top.enter_context(tc.tile_pool(name="const", bufs=1))
        persist = top.enter_context(tc.tile_pool(name="persist", bufs=1))
        ps_small = top.enter_context(tc.tile_pool(name="ps_small", bufs=2, space="PSUM"))
        ps_mid = top.enter_context(tc.tile_pool(name="ps_mid", bufs=2, space="PSUM"))
        ps_big = top.enter_context(tc.tile_pool(name="ps_big", bufs=2, space="PSUM"))

        def body():
            transposes_a = []   # phase A+ transpose matmul instructions
            dr_insts = []       # all DoubleRow matmuls
            transposes_e = []   # epilogue transposes

            # ---- constants ----
            identf = const.tile([128, 128], F32, tag="identf")
            make_identity(nc, identf[:])
            gam = const.tile([128, 1], F32, tag="gam")
            nc.sync.dma_start(gam[:], bass.AP(tensor=gamma, offset=0, ap=[[0, 128], [1, 1]]))
            bp1 = const.tile([128, 4], F32, tag="bp1")
            nc.sync.dma_start(bp1[:], b_p1.rearrange("(o p) -> p o", p=128))
            bp2 = const.tile([128, 4], F32, tag="bp2")
            nc.sync.dma_start(bp2[:], b_p2.rearrange("(o p) -> p o", p=128))
            bqt = const.tile([128, 4], F32, tag="bqt")
            nc.sync.dma_start(bqt[:], b_q.rearrange("(o p) -> p o", p=128))
            nc.vector.tensor_scalar_mul(bqt[:], bqt[:], 8.0)
            bkt = const.tile([128, 4], F32, tag="bkt")
            nc.sync.dma_start(bkt[:], b_k.rearrange("(o p) -> p o", p=128))
            nc.vector.tensor_scalar_mul(bkt[:], bkt[:], 8.0)
            bvr = const.tile([128, D], F32, tag="bvr")
            nc.sync.dma_start(bvr[:], bass.AP(tensor=b_v, offset=0, ap=[[0, 128], [1, D]]))
            nc.vector.tensor_scalar_mul(bvr[:], bvr[:], 8.0)
            gam8 = const.tile([128, 1], F32, tag="gam8")
            nc.vector.tensor_scalar_mul(gam8[:], gam[:], 0.125)
            ones8 = const.tile([128, 2, 128], FP8, tag="ones8")
            nc.vector.memset(ones8[:], 1.0)
            sep8a = const.tile([128, 2, 16], FP8, tag="sep8a")
            nc.vector.memset(sep8a[:], 0.0)

            hsumT = persist.tile([128, 4, S], F32, tag="hsumT")
            fT = persist.tile([128, 4, S], F32, tag="fT")

            # ================= phase A: h projections =================
            phA = ExitStack()
            h8pool = phA.enter_context(tc.tile_pool(name="h8", bufs=1))
            h1_8 = h8pool.tile([128, 4, S], FP8, tag="h1_8")
            h2_8 = h8pool.tile([128, 4, S], FP8, tag="h2_8")
            with ExitStack() as phAw:
                wp = phAw.enter_context(tc.tile_pool(name="wp", bufs=1))
                xa = phAw.enter_context(tc.tile_pool(name="xa", bufs=2))
                xb = phAw.enter_context(tc.tile_pool(name="xb", bufs=1))
                hf = phAw.enter_context(tc.tile_pool(name="hf", bufs=2))

                Wp1_s = wp.tile([128, 6, D], F32R, tag="Wp1")
                nc.sync.dma_start(Wp1_s[:], W_p1.rearrange("(ko p) d -> p ko d", p=128))
                Wp2_s = wp.tile([128, 8, D], F32R, tag="Wp2")
                nc.sync.dma_start(Wp2_s[:], W_p2.rearrange("(ko p) d -> p ko d", p=128))

                for side, (x_d, K, Wp_s, bt, h_8) in enumerate((
                    (x1, K1, Wp1_s, bp1, h1_8),
                    (x2, K2, Wp2_s, bp2, h2_8),
                )):
                    KC = K // 128
                    for sc in range(4):  # 512-wide s chunks
                        xin = xa.tile([128, 4, K2], F32, tag="xin")
                        nc.sync.dma_start(
                            xin[:, :, :K],
                            x_d[sc * 512:(sc + 1) * 512, :].rearrange(
                                "(ss p) k -> p ss k", p=128),
                        )
                        xT = xb.tile([128, 8, 512], F32R, tag="xT")
                        for kc in range(KC):
                            pt = ps_small.tile([128, 512], F32, tag="tp")
                            for ss in range(4):
                                mt = nc.tensor.matmul(
                                    pt[:, ss * 128:(ss + 1) * 128],
                                    xin[:, ss, kc * 128:(kc + 1) * 128],
                                    identf[:], is_transpose=True,
                                    start=True, stop=True)
                                transposes_a.append(mt)
                            nc.scalar.copy(xT[:, kc, :], pt[:])
                        for ds in range(4):
                            ph = ps_mid.tile([128, 512], F32, tag="mid")
                            for kc in range(KC):
                                nc.tensor.matmul(
                                    ph[:], Wp_s[:, kc, ds * 128:(ds + 1) * 128],
                                    xT[:, kc, :],
                                    start=(kc == 0), stop=(kc == KC - 1))
                            hslice = slice(sc * 512, (sc + 1) * 512)
                            if side == 0:
                                # h1 -> hsumT (fp32) and h1_8 (fp8)
                                nc.vector.tensor_scalar_add(
                                    hsumT[:, ds, hslice], ph[:], bt[:, ds:ds + 1])
                                nc.vector.tensor_copy(
                                    h1_8[:, ds, hslice], hsumT[:, ds, hslice])
                            else:
                                t2 = hf.tile([128, 512], F32, tag="t2")
                                nc.vector.tensor_scalar_add(
                                    t2[:], ph[:], bt[:, ds:ds + 1])
                                nc.vector.tensor_copy(h2_8[:, ds, hslice], t2[:])
                                nc.vector.tensor_add(
                                    hsumT[:, ds, hslice],
                                    hsumT[:, ds, hslice], t2[:])

            # ---- fence 1: plain fp8 matmul between transposes and DR ----
            ps_dmy = ps_mid.tile([128, 512], F32, tag="mid")
            fence1 = nc.tensor.matmul(ps_dmy[:, :16], ones8[:, 0, :],
                                      sep8a[:, 0, :], start=True, stop=True)
            for t in transposes_a:
                add_dep_helper(fence1.ins, t.ins, reason="fence transposes before DR")

            # ================= phase B: q/k/v projections (fp8 DR) ==========
            phBC = ExitStack()
            kvpool = phBC.enter_context(tc.tile_pool(name="kvpool", bufs=1))
            q1T = kvpool.tile([128, 4, S], FP8, tag="q1T")
            q2T = kvpool.tile([128, 4, S], FP8, tag="q2T")
            k1T = kvpool.tile([128, 4, S], FP8, tag="k1T")
            k2T = kvpool.tile([128, 4, S], FP8, tag="k2T")
            v1 = kvpool.tile([128, 16, D], FP8, tag="v1")
            v2 = kvpool.tile([128, 16, D], FP8, tag="v2")
            with ExitStack() as phB:
                wkv = phB.enter_context(tc.tile_pool(name="wkv", bufs=1))
                Wq_s = wkv.tile([128, 4, D], FP8, tag="Wq")
                Wk_s = wkv.tile([128, 4, D], FP8, tag="Wk")
                Wv_s = wkv.tile([128, 4, D], FP8, tag="Wv")
                wtmp = wkv.tile([128, 4, D], F32, tag="wtmp")
                for Wd, Ws in ((W_q, Wq_s), (W_k, Wk_s), (W_v, Wv_s)):
                    nc.sync.dma_start(wtmp[:], Wd.rearrange("(ko p) d -> p ko d", p=128))
                    nc.vector.tensor_scalar_mul(Ws[:], wtmp[:], 8.0)

                # qT / kT: [d', s] = W.T @ h
                for Ws, bt, dsts in ((Wq_s, bqt, (q1T, q2T)), (Wk_s, bkt, (k1T, k2T))):
                    for h_8, dst in zip((h1_8, h2_8), dsts):
                        for sc in range(4):
                            for ds in range(4):
                                ph = ps_mid.tile([128, 512], F32, tag="mid")
                                for c in range(2):
                                    mm = nc.tensor.matmul(
                                        ph[:],
                                        Ws[:, 2 * c:2 * c + 2, ds * 128:(ds + 1) * 128],
                                        h_8[:, 2 * c:2 * c + 2, sc * 512:(sc + 1) * 512],
                                        start=(c == 0), stop=(c == 1), perf_mode=DR)
                                    dr_insts.append(mm)
                                nc.scalar.add(
                                    dst[:, ds, sc * 512:(sc + 1) * 512], ph[:],
                                    bt[:, ds:ds + 1])
                # v: [s, d] = h.T @ W_v
                for h_8, vv in ((h1_8, v1), (h2_8, v2)):
                    for ss in range(16):
                        ph = ps_mid.tile([128, 512], F32, tag="mid")
                        for c in range(2):
                            mm = nc.tensor.matmul(
                                ph[:],
                                h_8[:, 2 * c:2 * c + 2, ss * 128:(ss + 1) * 128],
                                Wv_s[:, 2 * c:2 * c + 2, :],
                                start=(c == 0), stop=(c == 1), perf_mode=DR)
                            dr_insts.append(mm)
                        nc.vector.tensor_add(vv[:, ss, :], bvr[:], ph[:])

            # ================= phase C: attention (fp8 DR) =================
            with ExitStack() as phC:
                pr = phC.enter_context(tc.tile_pool(name="pr", bufs=2))
                sm = phC.enter_context(tc.tile_pool(name="sm", bufs=4))
                fo = phC.enter_context(tc.tile_pool(name="fo", bufs=3))

                for j in range(2):  # 1024-wide q tiles
                    J = slice(j * 1024, (j + 1) * 1024)
                    for d_i, (qT, kT, vv) in enumerate(
                            ((q1T, k2T, v2), (q2T, k1T, v1))):
                        pT8 = pr.tile([128, 16, 1024], FP8, tag="pT8")
                        # scoresT + exp
                        for kc in range(16):
                            psS = ps_big.tile([128, 1024], F32, tag="big")
                            for qh in range(2):
                                for c in range(2):
                                    mm = nc.tensor.matmul(
                                        psS[:, qh * 512:(qh + 1) * 512],
                                        kT[:, 2 * c:2 * c + 2, kc * 128:(kc + 1) * 128],
                                        qT[:, 2 * c:2 * c + 2,
                                           j * 1024 + qh * 512:j * 1024 + (qh + 1) * 512],
                                        start=(c == 0), stop=(c == 1), perf_mode=DR)
                                    dr_insts.append(mm)
                            nc.scalar.activation(pT8[:, kc, :], psS[:], AF.Exp,
                                                 scale=SCALE / 64.0)
                        # row sums (replicated across partitions) + inv
                        for qh in range(2):
                            psSum = ps_mid.tile([128, 512], F32, tag="mid")
                            for c8 in range(8):
                                mm = nc.tensor.matmul(
                                    psSum[:], ones8[:],
                                    pT8[:, 2 * c8:2 * c8 + 2,
                                        qh * 512:(qh + 1) * 512],
                                    start=(c8 == 0), stop=(c8 == 7), perf_mode=DR)
                                dr_insts.append(mm)
                            inv = sm.tile([128, 512], F32, tag="inv")
                            nc.vector.reciprocal(inv[:], psSum[:])
                            nc.vector.tensor_scalar_mul(inv[:], inv[:], gam8[:, 0:1])
                            # fused PV for this q-half
                            for ds in range(4):
                                psF = ps_mid.tile([128, 512], F32, tag="mid")
                                for c8 in range(8):
                                    mm = nc.tensor.matmul(
                                        psF[:],
                                        vv[:, 2 * c8:2 * c8 + 2, ds * 128:(ds + 1) * 128],
                                        pT8[:, 2 * c8:2 * c8 + 2,
                                            qh * 512:(qh + 1) * 512],
                                        start=(c8 == 0), stop=(c8 == 7), perf_mode=DR)
                                    dr_insts.append(mm)
                                fslice = slice(j * 1024 + qh * 512,
                                               j * 1024 + (qh + 1) * 512)
                                if d_i == 0:
                                    nc.vector.tensor_mul(
                                        fT[:, ds, fslice], psF[:], inv[:])
                                else:
                                    t2 = fo.tile([128, 512], F32, tag="t2")
                                    nc.vector.tensor_mul(t2[:], psF[:], inv[:])
                                    nc.vector.tensor_add(
                                        fT[:, ds, fslice],
                                        fT[:, ds, fslice], t2[:])
            phBC.close()
            phA.close()

            # ---- fence 2: plain fp8 matmul between DR and epilogue ----
            ps_dmy2 = ps_mid.tile([128, 512], F32, tag="mid")
            fence2 = nc.tensor.matmul(ps_dmy2[:, :16], ones8[:, 0, :],
                                      sep8a[:, 0, :], start=True, stop=True)
            for m in dr_insts:
                add_dep_helper(fence2.ins, m.ins, reason="fence DR before epilogue transposes")

            # ========== epilogue: out = transpose(fT + hsumT) ==========
            with ExitStack() as phE:
                eo = phE.enter_context(tc.tile_pool(name="eo", bufs=3))
                for ss in range(16):  # 128-row output tiles
                    oT = eo.tile([128, 4, 128], F32, tag="oT")
                    for ds in range(4):
                        nc.vector.tensor_add(
                            oT[:, ds, :],
                            fT[:, ds, ss * 128:(ss + 1) * 128],
                            hsumT[:, ds, ss * 128:(ss + 1) * 128])
                    psO = ps_mid.tile([128, 512], F32, tag="mid")
                    for ds in range(4):
                        mt = nc.tensor.matmul(
                            psO[:, ds * 128:(ds + 1) * 128],
                            oT[:, ds, :], identf[:],
                            is_transpose=True, start=True, stop=True)
                        transposes_e.append(mt)
                        add_dep_helper(mt.ins, fence2.ins, reason="epilogue after fence2")
                    o_tile = eo.tile([128, 512], F32, tag="o")
                    nc.vector.tensor_copy(o_tile[:], psO[:])
                    nc.sync.dma_start(out[ss * 128:(ss + 1) * 128, :], o_tile[:])

            # ensure all DR matmuls are after fence1
            for m in dr_insts:
                add_dep_helper(m.ins, fence1.ins, reason="DR after fence1")

        if reps is None:
            body()
        else:
            import concourse.mybir as _mybir
            with tc.For_i(0, reps, 1, hint_engines=tuple(_mybir.ALL_ENGINES)):
                body()

    nc.compile()
    _verify_pe_order(nc)
    _BUILD_CACHE[reps] = nc
    return nc


def _verify_pe_order(nc):
    """Walk final PE instruction order; assert no transpose directly adjacent
    to a DoubleRow matmul (hardware mode-transition hazard)."""
    import concourse.mybir as mybir
    for blk in nc.m.functions[0].blocks:
        prev_kind = None
        for inst in blk.instructions:
            if getattr(inst, "engine", None) != mybir.EngineType.PE:
                continue
            tn = type(inst).__name__
            if tn not in ("InstMatmult", "InstLdweights"):
                continue
            if getattr(inst, "is_transpose", False):
                kind = "tp"
            elif getattr(inst, "perf_mode", None) is not None:
                kind = "dr"
            else:
                kind = "plain"
            if {prev_kind, kind} == {"tp", "dr"}:
                raise AssertionError(
                    f"PE order hazard: {prev_kind} -> {kind} at {inst.name} "
                    f"in block {blk.name}")
            prev_kind = kind


def kernel(**inputs):
    from concourse.bass_utils import run_bass_kernel_spmd

    nc = build_bass(None)
    arrs = {k: np.asarray(v, dtype=np.float32) for k, v in inputs.items()}
    shared = {k: arrs[k] for k in
              ("W_p1", "W_p2", "W_q", "W_k", "W_v",
               "b_p1", "b_p2", "b_q", "b_k", "b_v", "gamma")}
    in_maps = [
        {"x1": arrs["x1"][b], "x2": arrs["x2"][b], **shared}
        for b in range(N_CORES)
    ]
    res = run_bass_kernel_spmd(nc, in_maps, list(range(N_CORES)))
    return np.stack([res.results[b]["out"] for b in range(N_CORES)], axis=0)


# revision 15
# speedup vs baseline: 1.7551x; 1.4300x over previous
"""Bidirectional cross-attention Trainium2 Bass kernel (v2).

Data-parallel over batch: 8 NeuronCores, one batch element each (SPMD, no
collectives).  Per core:

  phase A (fp32/f32r, PE transposes allowed):
    xT = PE-transpose(x) chunks; h = x @ W_p + b_p via f32r matmuls.
    Stored as hsumT = h1T+h2T (fp32, for the residual epilogue) and
    h1/h2 in fp8e4 (feeds the attention branch, which is scaled by gamma).
  phase B (fp8e4 DoubleRow matmuls):
    q1T,q2T,k1T,k2T (transposed [d,s]) and v1,v2 (natural [s,d]) in fp8e4.
  phase C (fp8e4 DoubleRow):
    scoresT[kpos,q] accumulated in PSUM -> exp on ACT (scale folded) ->
    probsT fp8 directly (no transposes!); row sums via ones-matmul
    (replicated across partitions); fusedT[d,q] = v.T @ probsT;
    normalize by gamma/sum (DVE).
  epilogue (PE transposes):
    outT = fusedT_1 + fusedT_2 + hsumT; PE-transpose to natural; DMA out.

The PE has a hardware hazard: a DoubleRow matmul adjacent to a transpose
matmul wedges the exec unit.  All DR work is fenced from transpose work by
dummy plain matmuls with explicit scheduling edges, and build() verifies
the final PE instruction order has no unfenced transitions.

Softmax max-subtraction is omitted: scores = q.k/sqrt(512) with these
projection scales is O(1), far from exp overflow.
"""
import sys

if "/opt/trn_rl_repo" not in sys.path:
    sys.path.insert(0, "/opt/trn_rl_repo")

import numpy as np

S = 2048
D = 512
K1 = 768
K2 = 1024
N_CORES = 8
SCALE = 1.0 / float(np.sqrt(np.float32(D)))

_BUILD_CACHE = {}


def build_bass(reps=None):
    if reps in _BUILD_CACHE:
        return _BUILD_CACHE[reps]

    import concourse.bass as bass
    from concourse import bacc
    import concourse.mybir as mybir
    import concourse.tile as tile
    from concourse.tile import add_dep_helper
    from concourse.masks import make_identity
    from contextlib import ExitStack

    F32 = mybir.dt.float32
    F32R = mybir.dt.float32r
    FP8 = mybir.dt.float8e4
    AF = mybir.ActivationFunctionType
    DR = mybir.MatmulPerfMode.DoubleRow

    nc = bacc.Bacc(None, target_bir_lowering=False, debug=False)

    x1 = nc.dram_tensor("x1", [S, K1], F32, kind="ExternalInput")
    x2 = nc.dram_tensor("x2", [S, K2], F32, kind="ExternalInput")
    W_p1 = nc.dram_tensor("W_p1", [K1, D], F32R, kind="ExternalInput")
    W_p2 = nc.dram_tensor("W_p2", [K2, D], F32R, kind="ExternalInput")
    W_q = nc.dram_tensor("W_q", [D, D], F32, kind="ExternalInput")
    W_k = nc.dram_tensor("W_k", [D, D], F32, kind="ExternalInput")
    W_v = nc.dram_tensor("W_v", [D, D], F32, kind="ExternalInput")
    b_p1 = nc.dram_tensor("b_p1", [D], F32, kind="ExternalInput")
    b_p2 = nc.dram_tensor("b_p2", [D], F32, kind="ExternalInput")
    b_q = nc.dram_tensor("b_q", [D], F32, kind="ExternalInput")
    b_k = nc.dram_tensor("b_k", [D], F32, kind="ExternalInput")
    b_v = nc.dram_tensor("b_v", [D], F32, kind="ExternalInput")
    gamma = nc.dram_tensor("gamma", [1], F32, kind="ExternalInput")
    out = nc.dram_tensor("out", [S, D], F32, kind="ExternalOutput")

    with tile.TileContext(nc) as tc, ExitStack() as top:
        const = top.enter_context(tc.tile_pool(name="const", bufs=1))
        persist = top.enter_context(tc.tile_pool(name="persist", bufs=1))
        ps_small = top.enter_context(tc.tile_pool(name="ps_small", bufs=2, space="PSUM"))
        ps_mid = top.enter_context(tc.tile_pool(name="ps_mid", bufs=2, space="PSUM"))
        ps_big = top.enter_context(tc.tile_pool(name="ps_big", bufs=2, space="PSUM"))

        def body():
            transposes_a = []   # phase A+ transpose matmul instructions
            dr_insts = []       # all DoubleRow matmuls
            transposes_e = []   # epilogue transposes

            # ---- constants ----
            identf = const.tile([128, 128], F32, tag="identf")
            make_identity(nc, identf[:])
            gam = const.tile([128, 1], F32, tag="gam")
            nc.sync.dma_start(gam[:], bass.AP(tensor=gamma, offset=0, ap=[[0, 128], [1, 1]]))
            bp1 = const.tile([128, 4], F32, tag="bp1")
            nc.sync.dma_start(bp1[:], b_p1.rearrange("(o p) -> p o", p=128))
            bp2 = const.tile([128, 4], F32, tag="bp2")
            nc.sync.dma_start(bp2[:], b_p2.rearrange("(o p) -> p o", p=128))
            bqt = const.tile([128, 4], F32, tag="bqt")
            nc.sync.dma_start(bqt[:], b_q.rearrange("(o p) -> p o", p=128))
            nc.vector.tensor_scalar_mul(bqt[:], bqt[:], 8.0)
            bkt = const.tile([128, 4], F32, tag="bkt")
            nc.sync.dma_start(bkt[:], b_k.rearrange("(o p) -> p o", p=128))
            nc.vector.tensor_scalar_mul(bkt[:], bkt[:], 8.0)
            bvr = const.tile([128, D], F32, tag="bvr")
            nc.sync.dma_start(bvr[:], bass.AP(tensor=b_v, offset=0, ap=[[0, 128], [1, D]]))
            nc.vector.tensor_scalar_mul(bvr[:], bvr[:], 8.0)
            gam8 = const.tile([128, 1], F32, tag="gam8")
            nc.vector.tensor_scalar_mul(gam8[:], gam[:], 0.125)
            ones8 = const.tile([128, 2, 128], FP8, tag="ones8")
            nc.vector.memset(ones8[:], 1.0)
            sep8a = const.tile([128, 2, 16], FP8, tag="sep8a")
            nc.vector.memset(sep8a[:], 0.0)

            hsumT = persist.tile([128, 4, S], F32, tag="hsumT")
            fT = persist.tile([128, 4, S], F32, tag="fT")

            # ================= phase A: h projections =================
            phA = ExitStack()
            h8pool = phA.enter_context(tc.tile_pool(name="h8", bufs=1))
            h1_8 = h8pool.tile([128, 4, S], FP8, tag="h1_8")
            h2_8 = h8pool.tile([128, 4, S], FP8, tag="h2_8")
            with ExitStack() as phAw:
                wp = phAw.enter_context(tc.tile_pool(name="wp", bufs=1))
                xa = phAw.enter_context(tc.tile_pool(name="xa", bufs=2))
                xb = phAw.enter_context(tc.tile_pool(name="xb", bufs=1))
                hf = phAw.enter_context(tc.tile_pool(name="hf", bufs=2))

                Wp1_s = wp.tile([128, 6, D], F32R, tag="Wp1")
                nc.gpsimd.dma_start(Wp1_s[:], W_p1.rearrange("(ko p) d -> p ko d", p=128))
                Wp2_s = wp.tile([128, 8, D], F32R, tag="Wp2")
                nc.gpsimd.dma_start(Wp2_s[:], W_p2.rearrange("(ko p) d -> p ko d", p=128))

                for side, (x_d, K, Wp_s, bt, h_8) in enumerate((
                    (x1, K1, Wp1_s, bp1, h1_8),
                    (x2, K2, Wp2_s, bp2, h2_8),
                )):
                    KC = K // 128
                    for sc in range(4):  # 512-wide s chunks
                        xin = xa.tile([128, 4, K2], F32, tag="xin")
                        dma_eng = nc.gpsimd if sc % 2 else nc.sync
                        dma_eng.dma_start(
                            xin[:, :, :K],
                            x_d[sc * 512:(sc + 1) * 512, :].rearrange(
                                "(ss p) k -> p ss k", p=128),
                        )
                        xT = xb.tile([128, 8, 512], F32R, tag="xT")
                        for kc in range(KC):
                            pt = ps_small.tile([128, 512], F32, tag="tp")
                            for ss in range(4):
                                mt = nc.tensor.matmul(
                                    pt[:, ss * 128:(ss + 1) * 128],
                                    xin[:, ss, kc * 128:(kc + 1) * 128],
                                    identf[:], is_transpose=True,
                                    start=True, stop=True)
                                transposes_a.append(mt)
                            nc.scalar.copy(xT[:, kc, :], pt[:])
                        for ds in range(4):
                            ph = ps_mid.tile([128, 512], F32, tag="mid")
                            for kc in range(KC):
                                nc.tensor.matmul(
                                    ph[:], Wp_s[:, kc, ds * 128:(ds + 1) * 128],
                                    xT[:, kc, :],
                                    start=(kc == 0), stop=(kc == KC - 1))
                            hslice = slice(sc * 512, (sc + 1) * 512)
                            if side == 0:
                                # h1 -> hsumT (fp32) and h1_8 (fp8)
                                nc.vector.tensor_scalar_add(
                                    hsumT[:, ds, hslice], ph[:], bt[:, ds:ds + 1])
                                nc.vector.tensor_copy(
                                    h1_8[:, ds, hslice], hsumT[:, ds, hslice])
                            else:
                                t2 = hf.tile([128, 512], F32, tag="t2")
                                nc.vector.tensor_scalar_add(
                                    t2[:], ph[:], bt[:, ds:ds + 1])
                                nc.vector.tensor_copy(h2_8[:, ds, hslice], t2[:])
                                nc.vector.tensor_add(
                                    hsumT[:, ds, hslice],
                                    hsumT[:, ds, hslice], t2[:])

            # ---- fence 1: plain fp8 matmul between transposes and DR ----
            ps_dmy = ps_mid.tile([128, 512], F32, tag="mid")
            fence1 = nc.tensor.matmul(ps_dmy[:, :16], ones8[:, 0, :],
                                      sep8a[:, 0, :], start=True, stop=True)
            for t in transposes_a:
                add_dep_helper(fence1.ins, t.ins, reason="fence transposes before DR")

            # ================= phase B: q/k/v projections (fp8 DR) ==========
            phBC = ExitStack()
            kvpool = phBC.enter_context(tc.tile_pool(name="kvpool", bufs=1))
            q1T = kvpool.tile([128, 4, S], FP8, tag="q1T")
            q2T = kvpool.tile([128, 4, S], FP8, tag="q2T")
            k1T = kvpool.tile([128, 4, S], FP8, tag="k1T")
            k2T = kvpool.tile([128, 4, S], FP8, tag="k2T")
            v1 = kvpool.tile([128, 16, D], FP8, tag="v1")
            v2 = kvpool.tile([128, 16, D], FP8, tag="v2")
            with ExitStack() as phB:
                wkv = phB.enter_context(tc.tile_pool(name="wkv", bufs=1))
                Wq_s = wkv.tile([128, 4, D], FP8, tag="Wq")
                Wk_s = wkv.tile([128, 4, D], FP8, tag="Wk")
                Wv_s = wkv.tile([128, 4, D], FP8, tag="Wv")
                wtmp = wkv.tile([128, 4, D], F32, tag="wtmp")
                for Wd, Ws in ((W_q, Wq_s), (W_k, Wk_s), (W_v, Wv_s)):
                    nc.sync.dma_start(wtmp[:], Wd.rearrange("(ko p) d -> p ko d", p=128))
                    nc.vector.tensor_scalar_mul(Ws[:], wtmp[:], 8.0)

                def proj_T(Ws, bt, h_8, dst):
                    # [d', s] = W.T @ h
                    for sc in range(4):
                        for ds in range(4):
                            ph = ps_mid.tile([128, 512], F32, tag="mid")
                            for c in range(2):
                                mm = nc.tensor.matmul(
                                    ph[:],
                                    Ws[:, 2 * c:2 * c + 2, ds * 128:(ds + 1) * 128],
                                    h_8[:, 2 * c:2 * c + 2, sc * 512:(sc + 1) * 512],
                                    start=(c == 0), stop=(c == 1), perf_mode=DR)
                                dr_insts.append(mm)
                            nc.scalar.add(
                                dst[:, ds, sc * 512:(sc + 1) * 512], ph[:],
                                bt[:, ds:ds + 1])

                def proj_v(h_8, vv):
                    # [s, d] = h.T @ W_v
                    for ss in range(16):
                        ph = ps_mid.tile([128, 512], F32, tag="mid")
                        for c in range(2):
                            mm = nc.tensor.matmul(
                                ph[:],
                                h_8[:, 2 * c:2 * c + 2, ss * 128:(ss + 1) * 128],
                                Wv_s[:, 2 * c:2 * c + 2, :],
                                start=(c == 0), stop=(c == 1), perf_mode=DR)
                            dr_insts.append(mm)
                        nc.vector.tensor_add(vv[:, ss, :], bvr[:], ph[:])

                # direction-0 operands first so attention can start sooner
                proj_T(Wk_s, bkt, h2_8, k2T)
                proj_T(Wq_s, bqt, h1_8, q1T)
                proj_v(h2_8, v2)
                proj_T(Wk_s, bkt, h1_8, k1T)
                proj_T(Wq_s, bqt, h2_8, q2T)
                proj_v(h1_8, v1)

            # ================= phase C: attention (fp8 DR) =================
            with ExitStack() as phC:
                pr = phC.enter_context(tc.tile_pool(name="pr", bufs=2))
                sm = phC.enter_context(tc.tile_pool(name="sm", bufs=4))
                fo = phC.enter_context(tc.tile_pool(name="fo", bufs=3))

                for j in range(2):  # 1024-wide q tiles
                    J = slice(j * 1024, (j + 1) * 1024)
                    for d_i, (qT, kT, vv) in enumerate(
                            ((q1T, k2T, v2), (q2T, k1T, v1))):
                        pT8 = pr.tile([128, 16, 1024], FP8, tag="pT8")
                        # scoresT + exp
                        for kc in range(16):
                            psS = ps_big.tile([128, 1024], F32, tag="big")
                            for qh in range(2):
                                for c in range(2):
                                    mm = nc.tensor.matmul(
                                        psS[:, qh * 512:(qh + 1) * 512],
                                        kT[:, 2 * c:2 * c + 2, kc * 128:(kc + 1) * 128],
                                        qT[:, 2 * c:2 * c + 2,
                                           j * 1024 + qh * 512:j * 1024 + (qh + 1) * 512],
                                        start=(c == 0), stop=(c == 1), perf_mode=DR)
                                    dr_insts.append(mm)
                            nc.scalar.activation(pT8[:, kc, :], psS[:], AF.Exp,
                                                 scale=SCALE / 64.0)
                        # row sums (replicated across partitions) + inv
                        for qh in range(2):
                            psSum = ps_mid.tile([128, 512], F32, tag="mid")
                            for c8 in range(8):
                                mm = nc.tensor.matmul(
                                    psSum[:], ones8[:],
                                    pT8[:, 2 * c8:2 * c8 + 2,
                                        qh * 512:(qh + 1) * 512],
                                    start=(c8 == 0), stop=(c8 == 7), perf_mode=DR)
                                dr_insts.append(mm)
                            inv = sm.tile([128, 512], F32, tag="inv")
                            nc.vector.reciprocal(inv[:], psSum[:])
                            nc.vector.tensor_scalar_mul(inv[:], inv[:], gam8[:, 0:1])
                            # fused PV for this q-half
                            for ds in range(4):
                                psF = ps_mid.tile([128, 512], F32, tag="mid")
                                for c8 in range(8):
                                    mm = nc.tensor.matmul(
                                        psF[:],
                                        vv[:, 2 * c8:2 * c8 + 2, ds * 128:(ds + 1) * 128],
                                        pT8[:, 2 * c8:2 * c8 + 2,
                                            qh * 512:(qh + 1) * 512],
                                        start=(c8 == 0), stop=(c8 == 7), perf_mode=DR)
                                    dr_insts.append(mm)
                                fslice = slice(j * 1024 + qh * 512,
                                               j * 1024 + (qh + 1) * 512)
                                if d_i == 0:
                                    nc.vector.tensor_mul(
                                        fT[:, ds, fslice], psF[:], inv[:])
                                else:
                                    t2 = fo.tile([128, 512], F32, tag="t2")
                                    nc.vector.tensor_mul(t2[:], psF[:], inv[:])
                                    nc.vector.tensor_add(
                                        fT[:, ds, fslice],
                                        fT[:, ds, fslice], t2[:])
            phBC.close()
            phA.close()

            # ---- fence 2: plain fp8 matmul between DR and epilogue ----
            ps_dmy2 = ps_mid.tile([128, 512], F32, tag="mid")
            fence2 = nc.tensor.matmul(ps_dmy2[:, :16], ones8[:, 0, :],
                                      sep8a[:, 0, :], start=True, stop=True)
            for m in dr_insts:
                add_dep_helper(fence2.ins, m.ins, reason="fence DR before epilogue transposes")

            # ========== epilogue: out = transpose(fT + hsumT) ==========
            with ExitStack() as phE:
                eo = phE.enter_context(tc.tile_pool(name="eo", bufs=3))
                for ss in range(16):  # 128-row output tiles
                    oT = eo.tile([128, 4, 128], F32, tag="oT")
                    for ds in range(4):
                        nc.vector.tensor_add(
                            oT[:, ds, :],
                            fT[:, ds, ss * 128:(ss + 1) * 128],
                            hsumT[:, ds, ss * 128:(ss + 1) * 128])
                    psO = ps_mid.tile([128, 512], F32, tag="mid")
                    for ds in range(4):
                        mt = nc.tensor.matmul(
                            psO[:, ds * 128:(ds + 1) * 128],
                            oT[:, ds, :], identf[:],
                            is_transpose=True, start=True, stop=True)
                        transposes_e.append(mt)
                        add_dep_helper(mt.ins, fence2.ins, reason="epilogue after fence2")
                    o_tile = eo.tile([128, 512], F32, tag="o")
                    nc.vector.tensor_copy(o_tile[:], psO[:])
                    nc.sync.dma_start(out[ss * 128:(ss + 1) * 128, :], o_tile[:])

            # ensure all DR matmuls are after fence1
            for m in dr_insts:
                add_dep_helper(m.ins, fence1.ins, reason="DR after fence1")

        if reps is None:
            body()
        else:
            import concourse.mybir as _mybir
            with tc.For_i(0, reps, 1, hint_engines=tuple(_mybir.ALL_ENGINES)):
                body()

    nc.compile()
    _verify_pe_order(nc)
    _BUILD_CACHE[reps] = nc
    return nc


def _verify_pe_order(nc):
    """Walk final PE instruction order; assert no transpose directly adjacent
    to a DoubleRow matmul (hardware mode-transition hazard)."""
    import concourse.mybir as mybir
    for blk in nc.m.functions[0].blocks:
        prev_kind = None
        for inst in blk.instructions:
            if getattr(inst, "engine", None) != mybir.EngineType.PE:
                continue
            tn = type(inst).__name__
            if tn not in ("InstMatmult", "InstLdweights"):
                continue
            if getattr(inst, "is_transpose", False):
                kind = "tp"
            elif getattr(inst, "perf_mode", None) is not None:
                kind = "dr"
            else:
                kind = "plain"
            if {prev_kind, kind} == {"tp", "dr"}:
                raise AssertionError(
                    f"PE order hazard: {prev_kind} -> {kind} at {inst.name} "
                    f"in block {blk.name}")
            prev_kind = kind


def kernel(**inputs):
    from concourse.bass_utils import run_bass_kernel_spmd

    nc = build_bass(None)
    arrs = {k: np.asarray(v, dtype=np.float32) for k, v in inputs.items()}
    shared = {k: arrs[k] for k in
              ("W_p1", "W_p2", "W_q", "W_k", "W_v",
               "b_p1", "b_p2", "b_q", "b_k", "b_v", "gamma")}
    in_maps = [
        {"x1": arrs["x1"][b], "x2": arrs["x2"][b], **shared}
        for b in range(N_CORES)
    ]
    res = run_bass_kernel_spmd(nc, in_maps, list(range(N_CORES)))
    return np.stack([res.results[b]["out"] for b in range(N_CORES)], axis=0)
